# revision 1
# baseline (speedup 1.0000x reference)
"""AttentionBlock (GroupNorm + single-head full attention + residual) on 8 trn2 cores.

Sharding: core i -> batch i//4, query strip (i%4)*1024 .. +1024. Each core
computes its batch's full K/V (duplicated across the 4 cores sharing the
batch) so no inter-core communication is needed. The host rotates each
core's copy of x so its query strip sits at token rows 0..1023 (group-norm
statistics and attention key-sums are permutation-invariant over tokens),
which lets one SPMD program serve all cores.

Pipeline (per core, all phases under one TileContext):
  P1: stream token-major x tiles (bf16): Square+ones-matmuls accumulate
      per-channel sum/sum-of-squares in PSUM (group-norm stats) while
      PE-transposing x into a resident channel-major XT; group stats are
      combined/redistributed with tiny PE matmuls and a Newton-Raphson
      refined rsqrt (ScalarE Sqrt alone is only ~4e-3 accurate).
  P2: per 512-token window: normalize XT -> h (affine per channel), then
      K^T / V / Q^T projections (Q pre-scaled by C^-0.5, q-bias folded into
      the PSUM-evacuation; k-bias dropped - softmax shift-invariant; v/proj
      biases folded into a rank-1 post-projection bias).
  P3: per 512-query block: scores computed TRANSPOSED (S^T[k,q]) so exp()
      writes P^T directly (no P transposes); softmax skips max-subtraction
      (logits are O(+-10) for normalized inputs; exact up to fp arithmetic);
      row-sums via a ones-vector matmul; P^T V accumulated bank-coherently
      in PSUM; row normalization deferred to the projection output (row
      scaling commutes through out @ wp); projection + residual inline.

Numerics: attention pipeline in bf16 (matmuls accumulate fp32 in PSUM),
softmax and residual path fp32, projection weights/operands float32r.
End-to-end absmax-relative error vs the fp32 reference: ~5.4e-4.
HAM warm-up/keep-alive dummy matmuls hold the PE clock at 2.4 GHz.
"""

import numpy as np
from contextlib import ExitStack

import concourse.bass as bass
import concourse.bacc as bacc
import concourse.tile as tile
from concourse import mybir
from concourse.bass_utils import run_bass_kernel_spmd

B, H, W, C = 2, 64, 64, 512
T = H * W                 # 4096 tokens per batch
NCORES = 8
QS = 1024                 # queries per core
GROUPS, GSIZE = 32, 16    # 8 groups per 128-channel chunk
EPS = 1e-5
SCALE = float(C) ** -0.5
F32 = mybir.dt.float32
F32R = mybir.dt.float32r
import os
if os.environ.get('KERNEL_MM_F32'):
    F32R = mybir.dt.float32
BF16 = mybir.dt.bfloat16
DT_ATT = F32R if os.environ.get('KERNEL_F32R') else BF16
NCH = C // 128            # 4 channel chunks
NW = T // 512             # 8 token windows per batch
NQW = QS // 512           # 2 query windows per core
NBLK = QS // 512          # 2 attention q-blocks of 512 queries
NSUB = 4                  # 128-query subtiles per block


def _r(ap):
    return ap.bitcast(F32R)


def _build():
    nc = bacc.Bacc(None, target_bir_lowering=False)

    DT_X = BF16 if DT_ATT == BF16 else F32
    xkv_h = nc.declare_dram_parameter("xkv", [T, C], DT_X, isOutput=False)
    xres_h = nc.declare_dram_parameter("xres", [QS, C], F32, isOutput=False)
    wq_h = nc.declare_dram_parameter("wq", [C, C], DT_ATT, isOutput=False)
    wk_h = nc.declare_dram_parameter("wk", [C, C], DT_ATT, isOutput=False)
    wv_h = nc.declare_dram_parameter("wv", [C, C], DT_ATT, isOutput=False)
    wp_h = nc.declare_dram_parameter("wp", [C, C], F32R, isOutput=False)
    bq_h = nc.declare_dram_parameter("bq", [C], F32, isOutput=False)
    bv_h = nc.declare_dram_parameter("bv", [C], F32, isOutput=False)
    bp_h = nc.declare_dram_parameter("bp", [C], F32, isOutput=False)
    gamma_h = nc.declare_dram_parameter("gamma", [C], F32, isOutput=False)
    beta_h = nc.declare_dram_parameter("beta", [C], F32, isOutput=False)
    ident_h = nc.declare_dram_parameter("ident", [128, 128], F32, isOutput=False)
    sel_h = nc.declare_dram_parameter("selmat", [32, 512], F32, isOutput=False)
    out_h = nc.declare_dram_parameter("out", [QS, C], F32, isOutput=True)

    with tile.TileContext(nc) as tc, ExitStack() as ctx:
        persist = ctx.enter_context(tc.tile_pool(name="persist", bufs=1))
        small = ctx.enter_context(tc.tile_pool(name="small", bufs=1))

        # ---- constants / weights ----
        ident = persist.tile([128, 128], F32, tag="ident", name="ident")
        nc.sync.dma_start(out=ident, in_=ident_h[:, :])
        identb_h = nc.declare_dram_parameter("identb", [128, 128], BF16, isOutput=False)
        identb = persist.tile([128, 128], BF16, tag="identb", name="identb")
        nc.sync.dma_start(out=identb, in_=identb_h[:, :])

        bigpool = ctx.enter_context(tc.tile_pool(name="bigpool", bufs=1))
        xt_t = [bigpool.tile([128, T], DT_X, tag=f"xt{j}", name=f"xt{j}") for j in range(NCH)]
        kt_t = [bigpool.tile([128, T], DT_ATT, tag=f"kt{j}", name=f"kt{j}") for j in range(NCH)]
        qts_t = [bigpool.tile([128, QS], DT_ATT, tag=f"qts{j}", name=f"qts{j}") for j in range(NCH)]
        v_big = bigpool.tile([128, T // 128, C], DT_ATT, tag="vbig", name="vbig")
        ctx2 = ExitStack()
        wpool = ctx2.enter_context(tc.tile_pool(name="wpool", bufs=1))
        wq_t = [wpool.tile([128, C], DT_ATT, tag=f"wq{j}", name=f"wq{j}") for j in range(NCH)]
        wk_t = [wpool.tile([128, C], DT_ATT, tag=f"wk{j}", name=f"wk{j}") for j in range(NCH)]
        wv_t = [wpool.tile([128, C], DT_ATT, tag=f"wv{j}", name=f"wv{j}") for j in range(NCH)]
        wp_t = [persist.tile([128, C], F32R, tag=f"wp{j}", name=f"wp{j}") for j in range(NCH)]
        for j in range(NCH):
            sl = slice(j * 128, (j + 1) * 128)
            nc.scalar.dma_start(out=wq_t[j], in_=wq_h[sl, :])
            nc.scalar.dma_start(out=wk_t[j], in_=wk_h[sl, :])
            nc.scalar.dma_start(out=wv_t[j], in_=wv_h[sl, :])
            nc.scalar.dma_start(out=wp_t[j], in_=wp_h[sl, :])

        # per-channel vectors as [128, NCH] (column j = channel chunk j)
        def vec_tile(h, name):
            t = small.tile([128, NCH], F32, tag=name)
            nc.scalar.dma_start(out=t, in_=h.rearrange("(a p) -> p a", p=128))
            return t

        gamma_sb = vec_tile(gamma_h, "gamma")
        beta_sb = vec_tile(beta_h, "beta")
        bq_sb = vec_tile(bq_h, "bq")
        bv_sb = vec_tile(bv_h, "bv")
        bp_row = small.tile([1, C], F32, tag="bprow", name="bprow")
        nc.scalar.dma_start(out=bp_row, in_=bp_h.rearrange("(a c) -> a c", a=1))

        sbq = small.tile([128, NCH], F32, tag="sbq", name="sbq")
        nc.vector.tensor_scalar_mul(sbq, bq_sb, SCALE)
        eps8 = small.tile([8, 1], F32, tag="eps8", name="eps8")
        nc.vector.memset(eps8, EPS)

        rinv_t = [small.tile([128, 1], F32, tag=f"rinv{s}", name=f"rinv{s}") for s in range(NSUB * NBLK)]

        scale_t = [small.tile([128, 1], F32, tag=f"gnsc{j}", name=f"gnsc{j}") for j in range(NCH)]
        bias_t = [small.tile([128, 1], F32, tag=f"gnbi{j}", name=f"gnbi{j}") for j in range(NCH)]

        # PE warm-up: ~4us of dummy matmuls so HAM unthrottles 1.2->2.4GHz
        warm_sb = small.tile([128, 512], BF16, tag="warm_sb", name="warm_sb")
        nc.vector.memset(warm_sb, 0.0)
        ones_x = small.tile([128, 1], DT_X, tag="ones_x", name="ones_x")
        nc.vector.memset(ones_x, 1.0)
        sel_sb = small.tile([32, 512], F32, tag="sel_sb", name="sel_sb")
        nc.sync.dma_start(out=sel_sb, in_=sel_h[:, :])


        ident_att = identb if DT_X == BF16 else ident

        ctxkeep = ExitStack()
        p1ps_keep = ctxkeep.enter_context(tc.tile_pool(name="keepps", bufs=1, space="PSUM"))

        def keepalive(n):
            for _ in range(n):
                kps = p1ps_keep.tile([128, 512], F32, tag="keep", name="keep", bufs=1)
                nc.tensor.matmul(kps, warm_sb[:, 0:128], warm_sb,
                                 start=True, stop=True)

        with tc.tile_pool(name="p1ps", bufs=1, space="PSUM") as p1ps, \
             tc.tile_pool(name="p1sb", bufs=2) as p1sb:
            keepalive(20)

            sums_ps = p1ps.tile([1, C], F32, tag="sums", name="sums", bufs=1)
            ssq_ps = p1ps.tile([1, C], F32, tag="ssq", name="ssq", bufs=1)
            # stream x tiles: stats matmuls + transpose into resident XT
            for ti in range(T // 128):
                xt = p1sb.tile([128, C], DT_X, tag="xtok", name="xtok", bufs=6)
                nc.sync.dma_start(out=xt, in_=xkv_h[ti * 128:(ti + 1) * 128, :])
                xsq = p1sb.tile([128, C], DT_X, tag="xsq", name="xsq", bufs=3)
                nc.scalar.activation(xsq, xt, mybir.ActivationFunctionType.Square)
                nc.tensor.matmul(sums_ps, ones_x, xt,
                                 start=(ti == 0), stop=(ti == T // 128 - 1))
                nc.tensor.matmul(ssq_ps, ones_x, xsq,
                                 start=(ti == 0), stop=(ti == T // 128 - 1))
                psx = p1ps.tile([128, 512], DT_X, tag="tp", name="tp", bufs=2)
                for j in range(NCH):
                    nc.tensor.transpose(
                        psx[:, j * 128:(j + 1) * 128],
                        xt[:, j * 128:(j + 1) * 128], ident_att)
                for j in range(NCH):
                    nc.any.tensor_copy(
                        xt_t[j][:, ti * 128:(ti + 1) * 128],
                        psx[:, j * 128:(j + 1) * 128])
            # ---- group statistics from the accumulated sums ----
            srow = p1sb.tile([1, C], F32, tag="srow", name="srow")
            nc.any.tensor_copy(srow, sums_ps)
            qrow = p1sb.tile([1, C], F32, tag="qrow", name="qrow")
            nc.any.tensor_copy(qrow, ssq_ps)
            NTOK = float(GSIZE * T)
            mean_g = p1sb.tile([1, GROUPS], F32, tag="mean_g", name="mean_g")
            nc.vector.tensor_reduce(
                out=mean_g, in_=srow.rearrange("p (g c) -> p g c", g=GROUPS),
                axis=mybir.AxisListType.X, op=mybir.AluOpType.add)
            nc.vector.tensor_scalar_mul(mean_g, mean_g, 1.0 / NTOK)
            m2_g = p1sb.tile([1, GROUPS], F32, tag="m2_g", name="m2_g")
            nc.vector.tensor_reduce(
                out=m2_g, in_=qrow.rearrange("p (g c) -> p g c", g=GROUPS),
                axis=mybir.AxisListType.X, op=mybir.AluOpType.add)
            nc.vector.tensor_scalar_mul(m2_g, m2_g, 1.0 / NTOK)
            msq = p1sb.tile([1, GROUPS], F32, tag="msq", name="msq")
            nc.vector.tensor_tensor(out=msq, in0=mean_g, in1=mean_g,
                                    op=mybir.AluOpType.mult)
            ve = p1sb.tile([1, GROUPS], F32, tag="ve", name="ve")
            nc.vector.tensor_tensor(out=ve, in0=m2_g, in1=msq,
                                    op=mybir.AluOpType.subtract)
            nc.vector.tensor_scalar_add(ve, ve, EPS)
            sd = p1sb.tile([1, GROUPS], F32, tag="sd", name="sd")
            nc.scalar.activation(sd, ve, mybir.ActivationFunctionType.Sqrt)
            y0 = p1sb.tile([1, GROUPS], F32, tag="y0", name="y0")
            nc.vector.reciprocal(y0, sd)
            t1 = p1sb.tile([1, GROUPS], F32, tag="t1", name="t1")
            nc.vector.tensor_tensor(out=t1, in0=ve, in1=y0,
                                    op=mybir.AluOpType.mult)
            nc.vector.tensor_tensor(out=t1, in0=t1, in1=y0,
                                    op=mybir.AluOpType.mult)
            nc.vector.tensor_scalar(out=t1, in0=t1, scalar1=-0.5, scalar2=1.5,
                                    op0=mybir.AluOpType.mult,
                                    op1=mybir.AluOpType.add)
            rstd_g = p1sb.tile([1, GROUPS], F32, tag="rstd_g", name="rstd_g")
            nc.vector.tensor_tensor(out=rstd_g, in0=y0, in1=t1,
                                    op=mybir.AluOpType.mult)
            # transpose [1,32] rows -> [32, 2] (mean | rstd) on partitions
            g2_ps = p1ps.tile([32, 2], F32, tag="g2", name="g2", bufs=1)
            nc.tensor.transpose(g2_ps[:, 0:1], mean_g, ident[0:1, 0:1])
            nc.tensor.transpose(g2_ps[:, 1:2], rstd_g, ident[0:1, 0:1])
            g2 = p1sb.tile([32, 2], F32, tag="g2sb", name="g2sb")
            nc.any.tensor_copy(g2, g2_ps)
            for j in range(NCH):
                bps = p1ps.tile([128, 2], F32, tag="bps", name="bps", bufs=1)
                nc.tensor.matmul(bps, sel_sb[:, j * 128:(j + 1) * 128], g2,
                                 start=True, stop=True)
                bc = p1sb.tile([128, 2], F32, tag="bc", name="bc")
                nc.scalar.copy(bc, bps)
                nc.vector.tensor_tensor(out=scale_t[j], in0=bc[:, 1:2],
                                        in1=gamma_sb[:, j:j + 1],
                                        op=mybir.AluOpType.mult)
                mt = p1sb.tile([128, 1], F32, tag="mt", name="mt")
                nc.vector.tensor_tensor(out=mt, in0=bc[:, 0:1], in1=scale_t[j],
                                        op=mybir.AluOpType.mult)
                nc.vector.tensor_tensor(out=bias_t[j], in0=beta_sb[:, j:j + 1],
                                        in1=mt, op=mybir.AluOpType.subtract)

        keepalive(8)

        # ================= P2: normalize windows -> K^T, V, Q^T =================
        with tc.tile_pool(name="p2ps", bufs=2, space="PSUM") as p2ps, \
             tc.tile_pool(name="p2sb", bufs=2) as p2sb:
            for w in range(NW):
                hw = []
                for j in range(NCH):
                    hwj = p2sb.tile([128, 512], DT_ATT, tag=f"hw{j}", name=f"hw{j}")
                    nc.vector.tensor_scalar(
                        out=hwj, in0=xt_t[j][:, w * 512:(w + 1) * 512],
                        scalar1=scale_t[j], scalar2=bias_t[j],
                        op0=mybir.AluOpType.mult, op1=mybir.AluOpType.add)
                    hw.append(hwj)
                for ck in range(NCH):
                    ps = p2ps.tile([128, 512], F32, tag="kvp", name="kvp")
                    for ci in range(NCH):
                        nc.tensor.matmul(
                            ps, wk_t[ci][:, ck * 128:(ck + 1) * 128],
                            hw[ci], start=(ci == 0), stop=(ci == NCH - 1))
                    nc.any.tensor_copy(kt_t[ck][:, w * 512:(w + 1) * 512], ps)
                for i in range(4):
                    ps = p2ps.tile([128, 512], F32, tag="kvp", name="kvp")
                    for ci in range(NCH):
                        nc.tensor.matmul(
                            ps, hw[ci][:, i * 128:(i + 1) * 128],
                            wv_t[ci], start=(ci == 0), stop=(ci == NCH - 1))
                    nc.any.tensor_copy(v_big[:, w * 4 + i, :], ps)
            for qw in range(NQW):
                hw = []
                for j in range(NCH):
                    hwj = p2sb.tile([128, 512], DT_ATT, tag=f"hw{j}", name=f"hw{j}")
                    nc.vector.tensor_scalar(
                        out=hwj, in0=xt_t[j][:, qw * 512:(qw + 1) * 512],
                        scalar1=scale_t[j], scalar2=bias_t[j],
                        op0=mybir.AluOpType.mult, op1=mybir.AluOpType.add)
                    hw.append(hwj)
                for cq in range(NCH):
                    ps = p2ps.tile([128, 512], F32, tag="kvp", name="kvp")
                    for ci in range(NCH):
                        nc.tensor.matmul(
                            ps, wq_t[ci][:, cq * 128:(cq + 1) * 128],
                            hw[ci], start=(ci == 0), stop=(ci == NCH - 1))
                    nc.scalar.activation(
                        qts_t[cq][:, qw * 512:(qw + 1) * 512], ps,
                        mybir.ActivationFunctionType.Identity,
                        bias=sbq[:, cq:cq + 1], scale=SCALE)
        ctxkeep.close()
        ctx2.close()

        # ================= P3: attention =================
        otspool = ctx.enter_context(tc.tile_pool(name="otspool", bufs=1))
        ots_t = [otspool.tile([128, NCH, 512], F32R, tag=f"ots{b}", name=f"ots{b}") for b in range(NBLK)]
        with tc.tile_pool(name="p3ps", bufs=1, space="PSUM") as p3ps, \
             tc.tile_pool(name="p3ot", bufs=1, space="PSUM") as p3ot, \
             tc.tile_pool(name="p3sb", bufs=1) as p3sb, \
             tc.tile_pool(name="p3ac", bufs=4) as p3ac:
            # bias vector bp' = bv @ wp + bp, broadcast to all partitions
            bvp = p3ps.tile([1, C], F32, tag="sc", name="bvp", bufs=3)
            for ci in range(NCH):
                nc.tensor.matmul(bvp, bv_sb[:, ci:ci + 1], wp_t[ci].bitcast(F32),
                                 start=(ci == 0), stop=(ci == NCH - 1))
            bpp = p3sb.tile([1, C], F32, tag="bpp", name="bpp")
            nc.vector.tensor_tensor(out=bpp, in0=bvp, in1=bp_row,
                                    op=mybir.AluOpType.add)
            bppb = p3sb.tile([128, C], F32, tag="bppb", name="bppb")
            nc.gpsimd.partition_broadcast(bppb, bpp[0:1, :])

            ones_b = p3sb.tile([128, 1], DT_ATT, tag="ones_b", name="ones_b")
            nc.vector.memset(ones_b, 1.0)

            for blk in range(NBLK):
                q0 = blk * 512
                ot_ps = p3ot.tile([128, NCH, 512], F32, tag="ot", name="ot", bufs=1)
                rs_ps = p3ot.tile([1, 512], F32, tag="rsum", name="rsum", bufs=1)
                ptws = []
                for w2 in range(T // 128):
                    st_ps = p3ps.tile([128, 512], F32, tag="sc", name="st_ps", bufs=3)
                    for cq in range(NCH):
                        nc.tensor.matmul(
                            st_ps, kt_t[cq][:, w2 * 128:(w2 + 1) * 128],
                            qts_t[cq][:, q0:q0 + 512],
                            start=(cq == 0), stop=(cq == NCH - 1))
                    ptw = p3sb.tile([128, 512], DT_ATT, tag="ptw", name="ptw", bufs=36)
                    nc.scalar.activation(ptw, st_ps,
                                         mybir.ActivationFunctionType.Exp)
                    ptws.append(ptw)
                # bank-coherent accumulation passes: rowsum bank, then one
                # pass per ot bank (avoids PSUM write-queue cycling)
                for w2 in range(T // 128):
                    nc.tensor.matmul(rs_ps, ones_b, ptws[w2],
                                     start=(w2 == 0), stop=(w2 == T // 128 - 1))
                rs_row = p3sb.tile([1, 512], F32, tag="rs_row", name="rs_row", bufs=2)
                nc.any.tensor_copy(rs_row, rs_ps)
                for sub in range(NSUB):
                    rt_ps = p3ps.tile([128, 1], F32, tag="sc", name="rt", bufs=3)
                    nc.tensor.transpose(
                        rt_ps, rs_row[0:1, sub * 128:(sub + 1) * 128],
                        ident[0:1, 0:1])
                    rr = p3ac.tile([128, 1], F32, tag="rr", name="rr")
                    nc.any.tensor_copy(rr, rt_ps)
                    nc.vector.reciprocal(rinv_t[blk * NSUB + sub], rr)
                for cv in range(NCH):
                    for w2 in range(T // 128):
                        nc.tensor.matmul(
                            ot_ps[:, cv, :],
                            v_big[:, w2, cv * 128:(cv + 1) * 128],
                            ptws[w2], start=(w2 == 0),
                            stop=(w2 == T // 128 - 1))
                    nc.any.tensor_copy(ots_t[blk][:, cv, :], ot_ps[:, cv, :])

                for sub in range(NSUB):
                    ti = blk * NSUB + sub
                    ps_p = p3ps.tile([128, C], F32, tag="sc", name="ps_p", bufs=3)
                    for cv in range(NCH):
                        nc.tensor.matmul(
                            ps_p, ots_t[blk][:, cv, sub * 128:(sub + 1) * 128],
                            wp_t[cv], start=(cv == 0), stop=(cv == NCH - 1))
                    xres = p3sb.tile([128, C], F32, tag="xres", name="xres", bufs=3)
                    nc.sync.dma_start(out=xres, in_=xres_h[ti * 128:(ti + 1) * 128, :])
                    tmp = p3sb.tile([128, C], F32, tag="tmp", name="tmp", bufs=3)
                    nc.vector.scalar_tensor_tensor(
                        out=tmp, in0=ps_p, scalar=rinv_t[ti], in1=xres,
                        op0=mybir.AluOpType.mult, op1=mybir.AluOpType.add)
                    fin = p3sb.tile([128, C], F32, tag="fin", name="fin", bufs=3)
                    nc.vector.tensor_tensor(out=fin, in0=tmp, in1=bppb,
                                            op=mybir.AluOpType.add)
                    nc.sync.dma_start(out=out_h[ti * 128:(ti + 1) * 128, :], in_=fin)

        # (projection inlined into the attention block loop above)

    nc.compile()
    return nc


_NC_CACHE = []





def prepare_in_maps(x, gamma, beta, wq, bq, wk, bk, wv, bv, wp, bp):
    import ml_dtypes
    x = np.ascontiguousarray(np.asarray(x, dtype=np.float32))
    sel = np.zeros((32, 512), np.float32)
    for j in range(4):
        for cl in range(128):
            sel[8 * j + cl // GSIZE, j * 128 + cl] = 1.0
    wdt = np.float32 if os.environ.get('KERNEL_F32R') else ml_dtypes.bfloat16
    common = {
        "wq": np.asarray(wq, wdt), "wk": np.asarray(wk, wdt),
        "wv": np.asarray(wv, wdt), "wp": np.asarray(wp, np.float32),
        "bq": np.asarray(bq, np.float32), "bv": np.asarray(bv, np.float32),
        "bp": np.asarray(bp, np.float32),
        "gamma": np.asarray(gamma, np.float32),
        "beta": np.asarray(beta, np.float32),
        "ident": np.eye(128, dtype=np.float32),
        "identb": np.eye(128, dtype=ml_dtypes.bfloat16),
        "selmat": sel,
    }
    xf = x.reshape(B, T, C)
    xdt = np.float32 if os.environ.get('KERNEL_F32R') else ml_dtypes.bfloat16
    xatt = np.ascontiguousarray(xf.astype(xdt))
    in_maps = []
    for core in range(NCORES):
        b, qoff = core // 4, (core % 4) * QS
        # rotate so this core's query strip is rows 0..QS-1 (attention and
        # group stats are permutation-invariant over tokens)
        in_maps.append({
            **common,
            "xkv": np.ascontiguousarray(np.roll(xatt[b], -qoff, axis=0)),
            "xres": np.ascontiguousarray(xf[b, qoff:qoff + QS]),
        })
    return in_maps


def kernel(x, gamma, beta, wq, bq, wk, bk, wv, bv, wp, bp):
    if not _NC_CACHE:
        _NC_CACHE.append(_build())
    nc = _NC_CACHE[0]
    in_maps = prepare_in_maps(x, gamma, beta, wq, bq, wk, bk, wv, bv, wp, bp)
    res = run_bass_kernel_spmd(nc, in_maps, list(range(NCORES)))
    out = np.empty((B, T, C), np.float32)
    for core in range(NCORES):
        b, qoff = core // 4, (core % 4) * QS
        out[b, qoff:qoff + QS] = res.results[core]["out"]
    return out.reshape(B, H, W, C)



# revision 5
# speedup vs baseline: 1.4706x; 1.4706x over previous
"""AttentionBlock (GroupNorm + single-head full attention + residual) on 8 trn2 cores.

Sharding: core i -> batch i//4, query strip (i%4)*1024 .. +1024. Each core
computes its batch's full K/V (duplicated across the 4 cores sharing the
batch) so no inter-core communication is needed. The host rotates each
core's copy of x so its query strip sits at token rows 0..1023 (group-norm
statistics and attention key-sums are permutation-invariant over tokens),
which lets one SPMD program serve all cores.

v2 changes vs the bf16 baseline (292us):
  - x arrives channel-major (host-side transpose): no PE transposes and no
    ones/Square stats matmuls. Group-norm stats come from DVE bn_stats/
    bn_aggr over the resident XT tiles; the tiny group-combine and
    per-channel redistribution use 8/128-partition matmuls.
  - The Q projection is folded into the K side on the host (when bq == 0):
    Z = h @ (wk wq^T * C^-0.5), so scores S^T = Z^T . h_q use raw
    normalized h on the query side. One less projection pass.
  - The attention core runs in fp8 e4m3 with DoubleRow double-pumped
    matmuls (2 contraction chunks per instruction): scores, exp row-sums
    and P^T.V. Z^T / V / h_q are quantized to e4m3 at PSUM evacuation.
    exp(s - 5) keeps P in e4m3 range (logits measured in [-7.5, 7.2];
    e4m3 covers [2e-3, 240]). Softmax normalization is deferred to the
    f32r projection output, so the fp8 rowsum/noise largely cancels.
  - Projections (Z, V) stay bf16; out-projection stays float32r.
End-to-end absmax-relative error vs the fp32 reference: ~4.6e-3 (numpy
model; tolerance 2e-2). HAM keep-alive matmuls hold the PE clock at 2.4GHz.
"""

import numpy as np
from contextlib import ExitStack

import concourse.bass as bass
import concourse.bacc as bacc
import concourse.tile as tile
from concourse import mybir
from concourse.bass_utils import run_bass_kernel_spmd

B, H, W, C = 2, 64, 64, 512
T = H * W                 # 4096 tokens per batch
NCORES = 8
QS = 1024                 # queries per core
GROUPS, GSIZE = 32, 16    # 8 groups per 128-channel chunk
EPS = 1e-5
SCALE = float(C) ** -0.5
SHIFT = 5.0               # softmax logit shift so exp() fits e4m3
F32 = mybir.dt.float32
F32R = mybir.dt.float32r
BF16 = mybir.dt.bfloat16
E4 = mybir.dt.float8e4
DR = mybir.MatmulPerfMode.DoubleRow
NCH = C // 128            # 4 channel chunks
NW = T // 512             # 8 token windows per batch
NBLK = QS // 512          # 2 attention q-blocks of 512 queries
NSUB = 4                  # 128-query subtiles per block
NKP = T // 256            # 16 key-tile pairs per q-block


def _build(fold_q: bool):
    nc = bacc.Bacc(None, target_bir_lowering=False)

    xt_h = nc.declare_dram_parameter("xt", [C, T], BF16, isOutput=False)
    xres_h = nc.declare_dram_parameter("xres", [QS, C], F32, isOutput=False)
    g_h = nc.declare_dram_parameter("gmat", [C, C], BF16, isOutput=False)
    wv_h = nc.declare_dram_parameter("wv", [C, C], BF16, isOutput=False)
    wp_h = nc.declare_dram_parameter("wp", [C, C], F32R, isOutput=False)
    bv_h = nc.declare_dram_parameter("bv", [C], F32, isOutput=False)
    bp_h = nc.declare_dram_parameter("bp", [C], F32, isOutput=False)
    gamma_h = nc.declare_dram_parameter("gamma", [C], F32, isOutput=False)
    beta_h = nc.declare_dram_parameter("beta", [C], F32, isOutput=False)
    sel8_h = nc.declare_dram_parameter("sel8", [128, 8], F32, isOutput=False)
    repl8_h = nc.declare_dram_parameter("repl8", [8, 128], F32, isOutput=False)
    if fold_q:
        wq_h = nc.declare_dram_parameter("wq", [C, C], BF16, isOutput=False)
        bq_h = nc.declare_dram_parameter("bq", [C], F32, isOutput=False)
    out_h = nc.declare_dram_parameter("out", [QS, C], F32, isOutput=True)

    with tile.TileContext(nc) as tc, ExitStack() as ctx:
        persist = ctx.enter_context(tc.tile_pool(name="persist", bufs=1))
        small = ctx.enter_context(tc.tile_pool(name="small", bufs=1))

        bigpool = ctx.enter_context(tc.tile_pool(name="bigpool", bufs=1))
        xt_t = [bigpool.tile([128, T], BF16, tag=f"xt{j}", name=f"xt{j}") for j in range(NCH)]
        # fp8 operand tiles in DoubleRow pair layout [128, 2, ...]
        zt2 = [bigpool.tile([128, 2, T], E4, tag=f"zt{c}", name=f"zt{c}") for c in range(2)]
        qts2 = [bigpool.tile([128, 2, QS], E4, tag=f"qts{c}", name=f"qts{c}") for c in range(2)]
        v_big = bigpool.tile([128, T // 128, C], E4, tag="vbig", name="vbig")

        ctx2 = ExitStack()
        wpool = ctx2.enter_context(tc.tile_pool(name="wpool", bufs=1))
        g_t = [wpool.tile([128, C], BF16, tag=f"g{j}", name=f"g{j}") for j in range(NCH)]
        wv_t = [wpool.tile([128, C], BF16, tag=f"wv{j}", name=f"wv{j}") for j in range(NCH)]
        wp_t = [persist.tile([128, C], F32R, tag=f"wp{j}", name=f"wp{j}") for j in range(NCH)]
        if fold_q:
            wq_t = [wpool.tile([128, C], BF16, tag=f"wq{j}", name=f"wq{j}") for j in range(NCH)]
        for j in range(NCH):
            sl = slice(j * 128, (j + 1) * 128)
            nc.scalar.dma_start(out=g_t[j], in_=g_h[sl, :])
            nc.scalar.dma_start(out=wv_t[j], in_=wv_h[sl, :])
            nc.scalar.dma_start(out=wp_t[j], in_=wp_h[sl, :])
            if fold_q:
                nc.scalar.dma_start(out=wq_t[j], in_=wq_h[sl, :])

        # per-channel vectors as [128, NCH] (column j = channel chunk j)
        def vec_tile(h, name):
            t = small.tile([128, NCH], F32, tag=name)
            nc.scalar.dma_start(out=t, in_=h.rearrange("(a p) -> p a", p=128))
            return t

        gamma_sb = vec_tile(gamma_h, "gamma")
        beta_sb = vec_tile(beta_h, "beta")
        bv_sb = vec_tile(bv_h, "bv")
        bp_row = small.tile([1, C], F32, tag="bprow", name="bprow")
        nc.scalar.dma_start(out=bp_row, in_=bp_h.rearrange("(a c) -> a c", a=1))
        sel8 = small.tile([128, 8], F32, tag="sel8", name="sel8")
        nc.sync.dma_start(out=sel8, in_=sel8_h[:, :])
        repl8 = small.tile([8, 128], F32, tag="repl8", name="repl8")
        nc.sync.dma_start(out=repl8, in_=repl8_h[:, :])
        if fold_q:
            bq_sb = vec_tile(bq_h, "bq")
            sbq = small.tile([128, NCH], F32, tag="sbq", name="sbq")
            nc.vector.tensor_scalar_mul(sbq, bq_sb, SCALE)

        ones1 = small.tile([1, 1], F32, tag="ones1", name="ones1")
        nc.vector.memset(ones1, 1.0)
        nshift = small.tile([128, 1], F32, tag="nshift", name="nshift")
        nc.vector.memset(nshift, -SHIFT)
        onesd = small.tile([128, 2, 16], E4, tag="onesd", name="onesd")
        nc.vector.memset(onesd, 1.0)

        rinv_t = [small.tile([128, 1], F32, tag=f"rinv{s}", name=f"rinv{s}") for s in range(NSUB * NBLK)]
        scale_t = [small.tile([128, 1], F32, tag=f"gnsc{j}", name=f"gnsc{j}") for j in range(NCH)]
        bias_t = [small.tile([128, 1], F32, tag=f"gnbi{j}", name=f"gnbi{j}") for j in range(NCH)]

        # PE warm-up / keep-alive dummy matmuls (HAM unthrottle 1.2->2.4GHz)
        warm_sb = small.tile([128, 512], BF16, tag="warm_sb", name="warm_sb")
        nc.vector.memset(warm_sb, 0.0)

        ctxkeep = ExitStack()
        p1ps_keep = ctxkeep.enter_context(tc.tile_pool(name="keepps", bufs=1, space="PSUM"))

        def keepalive(n, lhs=None):
            for _ in range(n):
                kps = p1ps_keep.tile([128, 512], F32, tag="keep", name="keep", bufs=1)
                if lhs is None:
                    nc.tensor.matmul(kps, warm_sb[:, 0:128], warm_sb,
                                     start=True, stop=True)
                else:
                    nc.tensor.matmul(kps[0:1, :], lhs, warm_sb,
                                     start=True, stop=True)

        # ================= P1: stream XT, bn_stats group statistics =========
        with tc.tile_pool(name="p1ps", bufs=1, space="PSUM") as p1ps, \
             tc.tile_pool(name="p1sb", bufs=1) as p1sb:
            keepalive(18)
            mv = []
            for j in range(NCH):
                nc.sync.dma_start(out=xt_t[j], in_=xt_h[j * 128:(j + 1) * 128, :])
                bns = p1sb.tile([128, NW, 6], F32, tag=f"bns{j}", name=f"bns{j}")
                for s in range(NW):
                    nc.vector.bn_stats(bns[:, s, :], xt_t[j][:, s * 512:(s + 1) * 512])
                mvj = p1sb.tile([128, 2], F32, tag=f"mv{j}", name=f"mv{j}")
                nc.vector.bn_aggr(mvj, bns)
                mv.append(mvj)
                # keep-alive burst gated on this chunk's stats (spaces the
                # dummy matmuls across P1 so HAM stays warm)
                wj = p1sb.tile([128, 1], BF16, tag=f"warm{j}", name=f"warm{j}")
                nc.any.tensor_copy(wj, mvj[:, 0:1])
                keepalive(3, lhs=wj)
            # S[128, 8]: col j = mean_c (chunk j), col 4+j = E[x^2]_c
            S = p1sb.tile([128, 8], F32, tag="S", name="S")
            for j in range(NCH):
                nc.any.tensor_copy(S[:, j:j + 1], mv[j][:, 0:1])
                msq = p1sb.tile([128, 1], F32, tag="msq", name="msq")
                nc.vector.tensor_tensor(out=msq, in0=mv[j][:, 0:1], in1=mv[j][:, 0:1],
                                        op=mybir.AluOpType.mult)
                nc.vector.tensor_tensor(out=S[:, 4 + j:5 + j], in0=mv[j][:, 1:2],
                                        in1=msq, op=mybir.AluOpType.add)
            # group-combine over the 16 channels of each group: [8, 8]
            g8_ps = p1ps.tile([8, 8], F32, tag="g8", name="g8", bufs=1)
            nc.tensor.matmul(g8_ps, sel8, S, start=True, stop=True)
            vals = p1sb.tile([8, 8], F32, tag="vals", name="vals")
            nc.vector.tensor_scalar_mul(vals, g8_ps, 1.0 / GSIZE)
            # vals cols 0-3 = group mean; compute rstd into cols 4-7
            msq8 = p1sb.tile([8, 4], F32, tag="msq8", name="msq8")
            nc.vector.tensor_tensor(out=msq8, in0=vals[:, 0:4], in1=vals[:, 0:4],
                                    op=mybir.AluOpType.mult)
            ve = p1sb.tile([8, 4], F32, tag="ve", name="ve")
            nc.vector.tensor_tensor(out=ve, in0=vals[:, 4:8], in1=msq8,
                                    op=mybir.AluOpType.subtract)
            nc.vector.tensor_scalar_add(ve, ve, EPS)
            sd = p1sb.tile([8, 4], F32, tag="sd", name="sd")
            nc.scalar.activation(sd, ve, mybir.ActivationFunctionType.Sqrt)
            y0 = p1sb.tile([8, 4], F32, tag="y0", name="y0")
            nc.vector.reciprocal(y0, sd)
            # one Newton-Raphson step: rstd = y0 * (1.5 - 0.5 * ve * y0^2)
            t1 = p1sb.tile([8, 4], F32, tag="t1", name="t1")
            nc.vector.tensor_tensor(out=t1, in0=ve, in1=y0, op=mybir.AluOpType.mult)
            nc.vector.tensor_tensor(out=t1, in0=t1, in1=y0, op=mybir.AluOpType.mult)
            nc.vector.tensor_scalar(out=t1, in0=t1, scalar1=-0.5, scalar2=1.5,
                                    op0=mybir.AluOpType.mult, op1=mybir.AluOpType.add)
            nc.vector.tensor_tensor(out=vals[:, 4:8], in0=y0, in1=t1,
                                    op=mybir.AluOpType.mult)
            # redistribute to per-channel [128, 8]
            b128_ps = p1ps.tile([128, 8], F32, tag="b128", name="b128", bufs=1)
            nc.tensor.matmul(b128_ps, repl8, vals, start=True, stop=True)
            bc = p1sb.tile([128, 8], F32, tag="bc", name="bc")
            nc.scalar.copy(bc, b128_ps)
            for j in range(NCH):
                nc.vector.tensor_tensor(out=scale_t[j], in0=bc[:, 4 + j:5 + j],
                                        in1=gamma_sb[:, j:j + 1],
                                        op=mybir.AluOpType.mult)
                mt = p1sb.tile([128, 1], F32, tag="mt", name="mt")
                nc.vector.tensor_tensor(out=mt, in0=bc[:, j:j + 1], in1=scale_t[j],
                                        op=mybir.AluOpType.mult)
                nc.vector.tensor_tensor(out=bias_t[j], in0=beta_sb[:, j:j + 1],
                                        in1=mt, op=mybir.AluOpType.subtract)

        keepalive(6)

        # ================= P2: normalize windows -> Z^T, V (+ Q^T) ==========
        with tc.tile_pool(name="p2ps", bufs=3, space="PSUM") as p2ps, \
             tc.tile_pool(name="p2sb", bufs=2) as p2sb:
            for w in range(NW):
                hw = []
                for j in range(NCH):
                    hwj = p2sb.tile([128, 512], BF16, tag=f"hw{j}", name=f"hw{j}")
                    nc.vector.tensor_scalar(
                        out=hwj, in0=xt_t[j][:, w * 512:(w + 1) * 512],
                        scalar1=scale_t[j], scalar2=bias_t[j],
                        op0=mybir.AluOpType.mult, op1=mybir.AluOpType.add)
                    hw.append(hwj)
                if w < NBLK and not fold_q:
                    # query-side operand is just normalized h in e4m3
                    for j in range(NCH):
                        nc.any.tensor_copy(
                            qts2[j // 2][:, j % 2, w * 512:(w + 1) * 512], hw[j])
                for ck in range(NCH):
                    ps = p2ps.tile([128, 512], F32, tag="kvp", name="kvp")
                    for ci in range(NCH):
                        nc.tensor.matmul(
                            ps, g_t[ci][:, ck * 128:(ck + 1) * 128],
                            hw[ci], start=(ci == 0), stop=(ci == NCH - 1))
                    nc.any.tensor_copy(
                        zt2[ck // 2][:, ck % 2, w * 512:(w + 1) * 512], ps)
                for i in range(4):
                    ps = p2ps.tile([128, 512], F32, tag="kvp", name="kvp")
                    for ci in range(NCH):
                        nc.tensor.matmul(
                            ps, hw[ci][:, i * 128:(i + 1) * 128],
                            wv_t[ci], start=(ci == 0), stop=(ci == NCH - 1))
                    nc.any.tensor_copy(v_big[:, w * 4 + i, :], ps)
                if w < NBLK and fold_q:
                    for cq in range(NCH):
                        ps = p2ps.tile([128, 512], F32, tag="kvp", name="kvp")
                        for ci in range(NCH):
                            nc.tensor.matmul(
                                ps, wq_t[ci][:, cq * 128:(cq + 1) * 128],
                                hw[ci], start=(ci == 0), stop=(ci == NCH - 1))
                        nc.scalar.activation(
                            qts2[cq // 2][:, cq % 2, w * 512:(w + 1) * 512], ps,
                            mybir.ActivationFunctionType.Identity,
                            bias=sbq[:, cq:cq + 1], scale=SCALE)
        ctxkeep.close()
        ctx2.close()

        # ================= P3: fp8 DoubleRow attention =======================
        otspool = ctx.enter_context(tc.tile_pool(name="otspool", bufs=1))
        ots_t = [otspool.tile([128, NCH, 512], F32R, tag=f"ots{b}", name=f"ots{b}") for b in range(NBLK)]
        with tc.tile_pool(name="p3ps", bufs=1, space="PSUM") as p3ps, \
             tc.tile_pool(name="p3ot", bufs=1, space="PSUM") as p3ot, \
             tc.tile_pool(name="p3sb", bufs=1) as p3sb, \
             tc.tile_pool(name="p3ac", bufs=4) as p3ac:
            # bias vector bp' = bv @ wp + bp, broadcast to all partitions
            bvp = p3ps.tile([1, C], F32, tag="sc", name="bvp", bufs=3)
            for ci in range(NCH):
                nc.tensor.matmul(bvp, bv_sb[:, ci:ci + 1], wp_t[ci].bitcast(F32),
                                 start=(ci == 0), stop=(ci == NCH - 1))
            bpp = p3sb.tile([1, C], F32, tag="bpp", name="bpp")
            nc.vector.tensor_tensor(out=bpp, in0=bvp, in1=bp_row,
                                    op=mybir.AluOpType.add)
            bppb = p3sb.tile([128, C], F32, tag="bppb", name="bppb")
            nc.gpsimd.partition_broadcast(bppb, bpp[0:1, :])

            for blk in range(NBLK):
                q0 = blk * 512
                ot_ps = p3ot.tile([128, NCH, 512], F32, tag="ot", name="ot", bufs=1)
                rs_ps = p3ot.tile([1, 512], F32, tag="rsum", name="rsum", bufs=1)
                pts = []
                ptcur = None
                for w2 in range(T // 128):
                    st_ps = p3ps.tile([128, 512], F32, tag="sc", name="st_ps", bufs=3)
                    for c2 in range(2):
                        nc.tensor.matmul(
                            st_ps, zt2[c2][:, :, w2 * 128:(w2 + 1) * 128],
                            qts2[c2][:, :, q0:q0 + 512],
                            start=(c2 == 0), stop=(c2 == 1), perf_mode=DR)
                    if w2 % 2 == 0:
                        ptcur = p3sb.tile([128, 2, 512], E4, tag="pt", name="pt",
                                          bufs=2 * NKP + 2)
                        pts.append(ptcur)
                    nc.scalar.activation(ptcur[:, w2 % 2, :], st_ps,
                                         mybir.ActivationFunctionType.Exp,
                                         bias=nshift)
                # exp row-sums (over keys) via ones-matmuls, bank-coherent
                for p in range(NKP):
                    nc.tensor.matmul(rs_ps, onesd[:, :, 0:1], pts[p],
                                     start=(p == 0), stop=(p == NKP - 1),
                                     perf_mode=DR)
                rs_row = p3sb.tile([1, 512], F32, tag="rs_row", name="rs_row", bufs=2)
                nc.any.tensor_copy(rs_row, rs_ps)
                for sub in range(NSUB):
                    rt_ps = p3ps.tile([128, 1], F32, tag="sc", name="rt", bufs=3)
                    nc.tensor.transpose(
                        rt_ps, rs_row[0:1, sub * 128:(sub + 1) * 128], ones1)
                    rr = p3ac.tile([128, 1], F32, tag="rr", name="rr")
                    nc.any.tensor_copy(rr, rt_ps)
                    nc.vector.reciprocal(rinv_t[blk * NSUB + sub], rr)
                for cv in range(NCH):
                    for p in range(NKP):
                        nc.tensor.matmul(
                            ot_ps[:, cv, :],
                            v_big[:, 2 * p:2 * p + 2, cv * 128:(cv + 1) * 128],
                            pts[p], start=(p == 0), stop=(p == NKP - 1),
                            perf_mode=DR)
                    nc.any.tensor_copy(ots_t[blk][:, cv, :], ot_ps[:, cv, :])

                for sub in range(NSUB):
                    ti = blk * NSUB + sub
                    ps_p = p3ps.tile([128, C], F32, tag="sc", name="ps_p", bufs=3)
                    for cv in range(NCH):
                        nc.tensor.matmul(
                            ps_p, ots_t[blk][:, cv, sub * 128:(sub + 1) * 128],
                            wp_t[cv], start=(cv == 0), stop=(cv == NCH - 1))
                    xres = p3sb.tile([128, C], F32, tag="xres", name="xres", bufs=3)
                    nc.sync.dma_start(out=xres, in_=xres_h[ti * 128:(ti + 1) * 128, :])
                    tmp = p3sb.tile([128, C], F32, tag="tmp", name="tmp", bufs=3)
                    nc.vector.scalar_tensor_tensor(
                        out=tmp, in0=ps_p, scalar=rinv_t[ti], in1=xres,
                        op0=mybir.AluOpType.mult, op1=mybir.AluOpType.add)
                    fin = p3sb.tile([128, C], F32, tag="fin", name="fin", bufs=3)
                    nc.vector.tensor_tensor(out=fin, in0=tmp, in1=bppb,
                                            op=mybir.AluOpType.add)
                    nc.sync.dma_start(out=out_h[ti * 128:(ti + 1) * 128, :], in_=fin)

    nc.compile()
    return nc


_NC_CACHE = {}


def prepare_in_maps(x, gamma, beta, wq, bq, wk, bk, wv, bv, wp, bp):
    import ml_dtypes
    BFh = ml_dtypes.bfloat16
    x = np.ascontiguousarray(np.asarray(x, dtype=np.float32))
    fold_q = bool(np.any(np.asarray(bq) != 0))
    sel8 = np.zeros((128, 8), np.float32)
    for p in range(128):
        sel8[p, p // GSIZE] = 1.0
    repl8 = np.ascontiguousarray(sel8[:, :].T)
    wkf = np.asarray(wk, np.float32)
    wqf = np.asarray(wq, np.float32)
    if fold_q:
        gmat = wkf.astype(BFh)
    else:
        gmat = ((wkf @ wqf.T) * SCALE).astype(BFh)
    common = {
        "gmat": gmat,
        "wv": np.asarray(wv, BFh),
        "wp": np.asarray(wp, np.float32),
        "bv": np.asarray(bv, np.float32),
        "bp": np.asarray(bp, np.float32),
        "gamma": np.asarray(gamma, np.float32),
        "beta": np.asarray(beta, np.float32),
        "sel8": sel8, "repl8": repl8,
    }
    if fold_q:
        common["wq"] = np.asarray(wq, BFh)
        common["bq"] = np.asarray(bq, np.float32)
    xf = x.reshape(B, T, C)
    in_maps = []
    for core in range(NCORES):
        b, qoff = core // 4, (core % 4) * QS
        # rotate so this core's query strip is rows 0..QS-1 (attention and
        # group stats are permutation-invariant over tokens), then go
        # channel-major for direct DMA into the resident XT tiles
        xr = np.roll(xf[b], -qoff, axis=0)
        in_maps.append({
            **common,
            "xt": np.ascontiguousarray(xr.T.astype(BFh)),
            "xres": np.ascontiguousarray(xf[b, qoff:qoff + QS]),
        })
    return in_maps, fold_q


def kernel(x, gamma, beta, wq, bq, wk, bk, wv, bv, wp, bp):
    in_maps, fold_q = prepare_in_maps(x, gamma, beta, wq, bq, wk, bk, wv, bv, wp, bp)
    if fold_q not in _NC_CACHE:
        _NC_CACHE[fold_q] = _build(fold_q)
    nc = _NC_CACHE[fold_q]
    res = run_bass_kernel_spmd(nc, in_maps, list(range(NCORES)))
    out = np.empty((B, T, C), np.float32)
    for core in range(NCORES):
        b, qoff = core // 4, (core % 4) * QS
        out[b, qoff:qoff + QS] = res.results[core]["out"]
    return out.reshape(B, H, W, C)


# revision 11
# speedup vs baseline: 1.5005x; 1.0204x over previous
"""AttentionBlock (GroupNorm + single-head full attention + residual) on 8 trn2 cores.

Sharding: core i -> batch i//4, query strip (i%4)*1024 .. +1024. Each core
computes its batch's full K/V (duplicated across the 4 cores sharing the
batch) so no inter-core communication is needed. The host rotates each
core's copy of x so its query strip sits at token rows 0..1023 (group-norm
statistics and attention key-sums are permutation-invariant over tokens),
which lets one SPMD program serve all cores.

v2 changes vs the bf16 baseline (292us):
  - x arrives channel-major (host-side transpose): no PE transposes and no
    ones/Square stats matmuls. Group-norm stats come from DVE bn_stats/
    bn_aggr over the resident XT tiles; the tiny group-combine and
    per-channel redistribution use 8/128-partition matmuls.
  - The Q projection is folded into the K side on the host (when bq == 0):
    Z = h @ (wk wq^T * C^-0.5), so scores S^T = Z^T . h_q use raw
    normalized h on the query side. One less projection pass.
  - The attention core runs in fp8 e4m3 with DoubleRow double-pumped
    matmuls (2 contraction chunks per instruction): scores, exp row-sums
    and P^T.V. Z^T / V / h_q are quantized to e4m3 at PSUM evacuation.
    exp(s - 5) keeps P in e4m3 range (logits measured in [-7.5, 7.2];
    e4m3 covers [2e-3, 240]). Softmax normalization is deferred to the
    f32r projection output, so the fp8 rowsum/noise largely cancels.
  - Projections (Z, V) stay bf16; out-projection stays float32r.
End-to-end absmax-relative error vs the fp32 reference: ~4.6e-3 (numpy
model; tolerance 2e-2). HAM keep-alive matmuls hold the PE clock at 2.4GHz.
"""

import numpy as np
from contextlib import ExitStack

import concourse.bass as bass
import concourse.bacc as bacc
import concourse.tile as tile
from concourse import mybir
from concourse.bass_utils import run_bass_kernel_spmd

B, H, W, C = 2, 64, 64, 512
T = H * W                 # 4096 tokens per batch
NCORES = 8
QS = 1024                 # queries per core
GROUPS, GSIZE = 32, 16    # 8 groups per 128-channel chunk
EPS = 1e-5
SCALE = float(C) ** -0.5
SHIFT = 5.0               # softmax logit shift so exp() fits e4m3
F32 = mybir.dt.float32
F32R = mybir.dt.float32r
BF16 = mybir.dt.bfloat16
E4 = mybir.dt.float8e4
DR = mybir.MatmulPerfMode.DoubleRow
NCH = C // 128            # 4 channel chunks
NW = T // 512             # 8 token windows per batch
NBLK = QS // 512          # 2 attention q-blocks of 512 queries
NSUB = 4                  # 128-query subtiles per block
NKP = T // 256            # 16 key-tile pairs per q-block


def _build(fold_q: bool):
    nc = bacc.Bacc(None, target_bir_lowering=False)

    xt_h = nc.declare_dram_parameter("xt", [C, T], BF16, isOutput=False)
    xres_h = nc.declare_dram_parameter("xres", [QS, C], F32, isOutput=False)
    g_h = nc.declare_dram_parameter("gmat", [C, C], BF16, isOutput=False)
    wv_h = nc.declare_dram_parameter("wv", [C, C], BF16, isOutput=False)
    wp_h = nc.declare_dram_parameter("wp", [C, C], F32R, isOutput=False)
    bv_h = nc.declare_dram_parameter("bv", [C], F32, isOutput=False)
    bp_h = nc.declare_dram_parameter("bp", [C], F32, isOutput=False)
    gamma_h = nc.declare_dram_parameter("gamma", [C], F32, isOutput=False)
    beta_h = nc.declare_dram_parameter("beta", [C], F32, isOutput=False)
    sel8_h = nc.declare_dram_parameter("sel8", [128, 8], F32, isOutput=False)
    repl8_h = nc.declare_dram_parameter("repl8", [8, 128], F32, isOutput=False)
    if fold_q:
        wq_h = nc.declare_dram_parameter("wq", [C, C], BF16, isOutput=False)
        bq_h = nc.declare_dram_parameter("bq", [C], F32, isOutput=False)
    out_h = nc.declare_dram_parameter("out", [QS, C], F32, isOutput=True)

    with tile.TileContext(nc) as tc, ExitStack() as ctx:
        persist = ctx.enter_context(tc.tile_pool(name="persist", bufs=1))
        small = ctx.enter_context(tc.tile_pool(name="small", bufs=1))

        bigpool = ctx.enter_context(tc.tile_pool(name="bigpool", bufs=1))
        xt_t = [bigpool.tile([128, T], BF16, tag=f"xt{j}", name=f"xt{j}") for j in range(NCH)]
        # fp8 operand tiles in DoubleRow pair layout [128, 2, ...]
        zt2 = [bigpool.tile([128, 2, T], E4, tag=f"zt{c}", name=f"zt{c}") for c in range(2)]
        qts2 = [bigpool.tile([128, 2, QS], E4, tag=f"qts{c}", name=f"qts{c}") for c in range(2)]
        v_big = bigpool.tile([128, T // 128, C], E4, tag="vbig", name="vbig")

        ctx2 = ExitStack()
        wpool = ctx2.enter_context(tc.tile_pool(name="wpool", bufs=1))
        g_t = [wpool.tile([128, C], BF16, tag=f"g{j}", name=f"g{j}") for j in range(NCH)]
        wv_t = [wpool.tile([128, C], BF16, tag=f"wv{j}", name=f"wv{j}") for j in range(NCH)]
        wp_t = [persist.tile([128, C], F32R, tag=f"wp{j}", name=f"wp{j}") for j in range(NCH)]
        if fold_q:
            wq_t = [wpool.tile([128, C], BF16, tag=f"wq{j}", name=f"wq{j}") for j in range(NCH)]
        for j in range(NCH):
            sl = slice(j * 128, (j + 1) * 128)
            nc.scalar.dma_start(out=g_t[j], in_=g_h[sl, :])
            nc.scalar.dma_start(out=wv_t[j], in_=wv_h[sl, :])
            nc.scalar.dma_start(out=wp_t[j], in_=wp_h[sl, :])
            if fold_q:
                nc.scalar.dma_start(out=wq_t[j], in_=wq_h[sl, :])

        # per-channel vectors as [128, NCH] (column j = channel chunk j)
        def vec_tile(h, name):
            t = small.tile([128, NCH], F32, tag=name)
            nc.scalar.dma_start(out=t, in_=h.rearrange("(a p) -> p a", p=128))
            return t

        gamma_sb = vec_tile(gamma_h, "gamma")
        beta_sb = vec_tile(beta_h, "beta")
        bv_sb = vec_tile(bv_h, "bv")
        bp_row = small.tile([1, C], F32, tag="bprow", name="bprow")
        nc.scalar.dma_start(out=bp_row, in_=bp_h.rearrange("(a c) -> a c", a=1))
        sel8 = small.tile([128, 8], F32, tag="sel8", name="sel8")
        nc.sync.dma_start(out=sel8, in_=sel8_h[:, :])
        repl8 = small.tile([8, 128], F32, tag="repl8", name="repl8")
        nc.sync.dma_start(out=repl8, in_=repl8_h[:, :])
        if fold_q:
            bq_sb = vec_tile(bq_h, "bq")
            sbq = small.tile([128, NCH], F32, tag="sbq", name="sbq")
            nc.vector.tensor_scalar_mul(sbq, bq_sb, SCALE)

        ones1 = small.tile([1, 1], F32, tag="ones1", name="ones1")
        nc.vector.memset(ones1, 1.0)
        nshift = small.tile([128, 1], F32, tag="nshift", name="nshift")
        nc.vector.memset(nshift, -SHIFT)
        onesd = small.tile([128, 2, 16], E4, tag="onesd", name="onesd")
        nc.vector.memset(onesd, 1.0)

        rinv_t = [small.tile([128, 1], F32, tag=f"rinv{s}", name=f"rinv{s}") for s in range(NSUB * NBLK)]
        scale_t = [small.tile([128, 1], F32, tag=f"gnsc{j}", name=f"gnsc{j}") for j in range(NCH)]
        bias_t = [small.tile([128, 1], F32, tag=f"gnbi{j}", name=f"gnbi{j}") for j in range(NCH)]

        # PE warm-up / keep-alive dummy matmuls (HAM unthrottle 1.2->2.4GHz)
        warm_sb = small.tile([128, 512], BF16, tag="warm_sb", name="warm_sb")
        nc.vector.memset(warm_sb, 0.0)

        ctxkeep = ExitStack()
        p1ps_keep = ctxkeep.enter_context(tc.tile_pool(name="keepps", bufs=1, space="PSUM"))

        def keepalive(n, lhs=None):
            for _ in range(n):
                kps = p1ps_keep.tile([128, 512], F32, tag="keep", name="keep", bufs=1)
                if lhs is None:
                    nc.tensor.matmul(kps, warm_sb[:, 0:128], warm_sb,
                                     start=True, stop=True)
                else:
                    nc.tensor.matmul(kps[0:1, :], lhs, warm_sb,
                                     start=True, stop=True)

        # ================= P1: stream XT, bn_stats group statistics =========
        # Per-chunk pipeline (a chunk's group scale/bias only depends on its
        # own 128 channels): half-chunk DMAs -> 3D bn_stats -> bn_aggr ->
        # tiny 8/128-partition matmuls for the 16-channel group combine and
        # per-channel redistribution.
        with tc.tile_pool(name="p1ps", bufs=1, space="PSUM") as p1ps, \
             tc.tile_pool(name="p1sb", bufs=1) as p1sb:
            keepalive(18)
            HT = T // 2
            for j in range(NCH):
                # bf16 bn_stats buffer: all-2-byte operands enable the DVE
                # 2x mode; count/mean/M2 at bf16 costs ~0.4% on the group
                # variance (absorbed by the 2e-2 tolerance)
                bns = p1sb.tile([128, NW, 6], BF16, tag=f"bns{j}", name=f"bns{j}")
                for hf in range(2):
                    sl = slice(hf * HT, (hf + 1) * HT)
                    nc.sync.dma_start(out=xt_t[j][:, sl],
                                      in_=xt_h[j * 128:(j + 1) * 128, sl])
                    for s in range(4):
                        so = hf * 4 + s
                        nc.vector.bn_stats(
                            bns[:, so, :], xt_t[j][:, so * 512:(so + 1) * 512])
                mvj = p1sb.tile([128, 2], F32, tag=f"mv{j}", name=f"mv{j}")
                nc.vector.bn_aggr(mvj, bns)
                # Sj: col 0 = mean_c, col 1 = E[x^2]_c
                Sj = p1sb.tile([128, 2], F32, tag=f"S{j}", name=f"S{j}")
                nc.vector.tensor_copy(Sj[:, 0:1], mvj[:, 0:1])
                msq = p1sb.tile([128, 1], F32, tag="msq", name="msq")
                nc.vector.tensor_tensor(out=msq, in0=mvj[:, 0:1], in1=mvj[:, 0:1],
                                        op=mybir.AluOpType.mult)
                nc.vector.tensor_tensor(out=Sj[:, 1:2], in0=mvj[:, 1:2],
                                        in1=msq, op=mybir.AluOpType.add)
                g8_ps = p1ps.tile([8, 2], F32, tag="g8", name="g8", bufs=2)
                nc.tensor.matmul(g8_ps, sel8, Sj, start=True, stop=True)
                vals = p1sb.tile([8, 2], F32, tag=f"vals{j}", name=f"vals{j}")
                nc.vector.tensor_scalar_mul(vals, g8_ps, 1.0 / GSIZE)
                msq8 = p1sb.tile([8, 1], F32, tag="msq8", name="msq8")
                nc.vector.tensor_tensor(out=msq8, in0=vals[:, 0:1], in1=vals[:, 0:1],
                                        op=mybir.AluOpType.mult)
                ve = p1sb.tile([8, 1], F32, tag="ve", name="ve")
                nc.vector.tensor_tensor(out=ve, in0=vals[:, 1:2], in1=msq8,
                                        op=mybir.AluOpType.subtract)
                nc.vector.tensor_scalar_add(ve, ve, EPS)
                sd = p1sb.tile([8, 1], F32, tag="sd", name="sd")
                nc.scalar.activation(sd, ve, mybir.ActivationFunctionType.Sqrt)
                y0 = p1sb.tile([8, 1], F32, tag="y0", name="y0")
                nc.vector.reciprocal(y0, sd)
                # one Newton-Raphson step: rstd = y0 * (1.5 - 0.5 * ve * y0^2)
                t1 = p1sb.tile([8, 1], F32, tag="t1", name="t1")
                nc.vector.tensor_tensor(out=t1, in0=ve, in1=y0, op=mybir.AluOpType.mult)
                nc.vector.tensor_tensor(out=t1, in0=t1, in1=y0, op=mybir.AluOpType.mult)
                nc.vector.tensor_scalar(out=t1, in0=t1, scalar1=-0.5, scalar2=1.5,
                                        op0=mybir.AluOpType.mult, op1=mybir.AluOpType.add)
                nc.vector.tensor_tensor(out=vals[:, 1:2], in0=y0, in1=t1,
                                        op=mybir.AluOpType.mult)
                b128_ps = p1ps.tile([128, 2], F32, tag="b128", name="b128", bufs=2)
                nc.tensor.matmul(b128_ps, repl8, vals, start=True, stop=True)
                bc = p1sb.tile([128, 2], F32, tag=f"bc{j}", name=f"bc{j}")
                nc.scalar.copy(bc, b128_ps)
                nc.vector.tensor_tensor(out=scale_t[j], in0=bc[:, 1:2],
                                        in1=gamma_sb[:, j:j + 1],
                                        op=mybir.AluOpType.mult)
                mt = p1sb.tile([128, 1], F32, tag="mt", name="mt")
                nc.vector.tensor_tensor(out=mt, in0=bc[:, 0:1], in1=scale_t[j],
                                        op=mybir.AluOpType.mult)
                nc.vector.tensor_tensor(out=bias_t[j], in0=beta_sb[:, j:j + 1],
                                        in1=mt, op=mybir.AluOpType.subtract)
                # keep-alive burst gated on this chunk's stats (spaces the
                # dummy matmuls across P1 so HAM stays warm)
                wj = p1sb.tile([128, 1], BF16, tag=f"warm{j}", name=f"warm{j}")
                nc.any.tensor_copy(wj, mvj[:, 0:1])
                keepalive(3, lhs=wj)

        keepalive(4)
        ctxkeep.close()

        # ================= P2: normalize windows -> Z^T, V (+ Q^T) ==========
        with tc.tile_pool(name="p2ps", bufs=3, space="PSUM") as p2ps, \
             tc.tile_pool(name="p2sb", bufs=2) as p2sb:
            for w in range(NW):
                hw = []
                for j in range(NCH):
                    hwj = p2sb.tile([128, 512], BF16, tag=f"hw{j}", name=f"hw{j}")
                    nc.vector.tensor_scalar(
                        out=hwj, in0=xt_t[j][:, w * 512:(w + 1) * 512],
                        scalar1=scale_t[j], scalar2=bias_t[j],
                        op0=mybir.AluOpType.mult, op1=mybir.AluOpType.add)
                    hw.append(hwj)
                if w < NBLK and not fold_q:
                    # query-side operand is just normalized h in e4m3
                    for j in range(NCH):
                        nc.vector.tensor_copy(
                            qts2[j // 2][:, j % 2, w * 512:(w + 1) * 512], hw[j])
                # double-bank PSUM pairs; one wide evacuation per pair,
                # Z -> DVE, V -> ScalarE (keeps both under the PE pace)
                for cp in range(2):
                    ps2 = p2ps.tile([128, 2, 512], F32, tag="zp", name="zp", bufs=2)
                    for hh in range(2):
                        ck = 2 * cp + hh
                        for ci in range(NCH):
                            nc.tensor.matmul(
                                ps2[:, hh, :], g_t[ci][:, ck * 128:(ck + 1) * 128],
                                hw[ci], start=(ci == 0), stop=(ci == NCH - 1))
                    nc.vector.tensor_copy(zt2[cp][:, :, w * 512:(w + 1) * 512], ps2)
                for vp in range(2):
                    ps2 = p2ps.tile([128, 2, 512], F32, tag="vp", name="vp", bufs=2)
                    for hh in range(2):
                        i = 2 * vp + hh
                        for ci in range(NCH):
                            nc.tensor.matmul(
                                ps2[:, hh, :], hw[ci][:, i * 128:(i + 1) * 128],
                                wv_t[ci], start=(ci == 0), stop=(ci == NCH - 1))
                    nc.scalar.copy(v_big[:, w * 4 + 2 * vp:w * 4 + 2 * vp + 2, :], ps2)
                if w < NBLK and fold_q:
                    for cq in range(NCH):
                        ps = p2ps.tile([128, 512], F32, tag="kvp", name="kvp")
                        for ci in range(NCH):
                            nc.tensor.matmul(
                                ps, wq_t[ci][:, cq * 128:(cq + 1) * 128],
                                hw[ci], start=(ci == 0), stop=(ci == NCH - 1))
                        nc.scalar.activation(
                            qts2[cq // 2][:, cq % 2, w * 512:(w + 1) * 512], ps,
                            mybir.ActivationFunctionType.Identity,
                            bias=sbq[:, cq:cq + 1], scale=SCALE)
        ctx2.close()

        # ================= P3: fp8 DoubleRow attention =======================
        otspool = ctx.enter_context(tc.tile_pool(name="otspool", bufs=1))
        ots_t = [otspool.tile([128, NCH, 512], F32R, tag=f"ots{b}", name=f"ots{b}") for b in range(NBLK)]
        with tc.tile_pool(name="p3ps", bufs=1, space="PSUM") as p3ps, \
             tc.tile_pool(name="p3ot", bufs=1, space="PSUM") as p3ot, \
             tc.tile_pool(name="p3sb", bufs=1) as p3sb, \
             tc.tile_pool(name="p3ac", bufs=4) as p3ac:
            # bias vector bp' = bv @ wp + bp, broadcast to all partitions
            bvp = p3ps.tile([1, C], F32, tag="sc", name="bvp", bufs=3)
            for ci in range(NCH):
                nc.tensor.matmul(bvp, bv_sb[:, ci:ci + 1], wp_t[ci].bitcast(F32),
                                 start=(ci == 0), stop=(ci == NCH - 1))
            bpp = p3sb.tile([1, C], F32, tag="bpp", name="bpp")
            nc.vector.tensor_tensor(out=bpp, in0=bvp, in1=bp_row,
                                    op=mybir.AluOpType.add)
            bppb = p3sb.tile([128, C], F32, tag="bppb", name="bppb")
            nc.gpsimd.partition_broadcast(bppb, bpp[0:1, :])

            for blk in range(NBLK):
                q0 = blk * 512
                ot_ps = p3ot.tile([128, NCH, 512], F32, tag="ot", name="ot", bufs=1)
                rs_ps = p3ot.tile([1, 512], F32, tag="rsum", name="rsum", bufs=1)
                pts = [None] * NKP

                # software-pipelined: scores/exp for key-pair p while the
                # rowsum + P^T.V matmuls consume pair p-1 (PE stays busy
                # instead of pacing on the ScalarE exp drain)
                def scores_pair(p):
                    for hh in range(2):
                        w2 = 2 * p + hh
                        st_ps = p3ps.tile([128, 512], F32, tag="sc", name="st_ps", bufs=3)
                        for c2 in range(2):
                            nc.tensor.matmul(
                                st_ps, zt2[c2][:, :, w2 * 128:(w2 + 1) * 128],
                                qts2[c2][:, :, q0:q0 + 512],
                                start=(c2 == 0), stop=(c2 == 1), perf_mode=DR)
                        if hh == 0:
                            pts[p] = p3sb.tile([128, 2, 512], E4, tag="pt",
                                               name="pt", bufs=8)
                        nc.scalar.activation(pts[p][:, hh, :], st_ps,
                                             mybir.ActivationFunctionType.Exp,
                                             bias=nshift)

                def consume_pair(p):
                    nc.tensor.matmul(rs_ps, onesd[:, :, 0:1], pts[p],
                                     start=(p == 0), stop=(p == NKP - 1),
                                     perf_mode=DR)
                    for cv in range(NCH):
                        nc.tensor.matmul(
                            ot_ps[:, cv, :],
                            v_big[:, 2 * p:2 * p + 2, cv * 128:(cv + 1) * 128],
                            pts[p], start=(p == 0), stop=(p == NKP - 1),
                            perf_mode=DR)

                for p in range(NKP + 1):
                    if p < NKP:
                        scores_pair(p)
                    if p >= 1:
                        consume_pair(p - 1)

                rs_row = p3sb.tile([1, 512], F32, tag="rs_row", name="rs_row", bufs=2)
                nc.any.tensor_copy(rs_row, rs_ps)
                for sub in range(NSUB):
                    rt_ps = p3ps.tile([128, 1], F32, tag="sc", name="rt", bufs=3)
                    nc.tensor.transpose(
                        rt_ps, rs_row[0:1, sub * 128:(sub + 1) * 128], ones1)
                    rr = p3ac.tile([128, 1], F32, tag="rr", name="rr")
                    nc.any.tensor_copy(rr, rt_ps)
                    nc.vector.reciprocal(rinv_t[blk * NSUB + sub], rr)
                nc.vector.tensor_copy(ots_t[blk][:, :, :], ot_ps[:, :, :])

                for sub in range(NSUB):
                    ti = blk * NSUB + sub
                    ps_p = p3ps.tile([128, C], F32, tag="sc", name="ps_p", bufs=3)
                    for cv in range(NCH):
                        nc.tensor.matmul(
                            ps_p, ots_t[blk][:, cv, sub * 128:(sub + 1) * 128],
                            wp_t[cv], start=(cv == 0), stop=(cv == NCH - 1))
                    xres = p3sb.tile([128, C], F32, tag="xres", name="xres", bufs=3)
                    nc.sync.dma_start(out=xres, in_=xres_h[ti * 128:(ti + 1) * 128, :])
                    tmp = p3sb.tile([128, C], F32, tag="tmp", name="tmp", bufs=3)
                    nc.vector.scalar_tensor_tensor(
                        out=tmp, in0=ps_p, scalar=rinv_t[ti], in1=xres,
                        op0=mybir.AluOpType.mult, op1=mybir.AluOpType.add)
                    fin = p3sb.tile([128, C], F32, tag="fin", name="fin", bufs=3)
                    nc.vector.tensor_tensor(out=fin, in0=tmp, in1=bppb,
                                            op=mybir.AluOpType.add)
                    nc.sync.dma_start(out=out_h[ti * 128:(ti + 1) * 128, :], in_=fin)

    nc.compile()
    return nc


_NC_CACHE = {}


def prepare_in_maps(x, gamma, beta, wq, bq, wk, bk, wv, bv, wp, bp):
    import ml_dtypes
    BFh = ml_dtypes.bfloat16
    x = np.ascontiguousarray(np.asarray(x, dtype=np.float32))
    fold_q = bool(np.any(np.asarray(bq) != 0))
    sel8 = np.zeros((128, 8), np.float32)
    for p in range(128):
        sel8[p, p // GSIZE] = 1.0
    repl8 = np.ascontiguousarray(sel8[:, :].T)
    wkf = np.asarray(wk, np.float32)
    wqf = np.asarray(wq, np.float32)
    if fold_q:
        gmat = wkf.astype(BFh)
    else:
        gmat = ((wkf @ wqf.T) * SCALE).astype(BFh)
    common = {
        "gmat": gmat,
        "wv": np.asarray(wv, BFh),
        "wp": np.asarray(wp, np.float32),
        "bv": np.asarray(bv, np.float32),
        "bp": np.asarray(bp, np.float32),
        "gamma": np.asarray(gamma, np.float32),
        "beta": np.asarray(beta, np.float32),
        "sel8": sel8, "repl8": repl8,
    }
    if fold_q:
        common["wq"] = np.asarray(wq, BFh)
        common["bq"] = np.asarray(bq, np.float32)
    xf = x.reshape(B, T, C)
    in_maps = []
    for core in range(NCORES):
        b, qoff = core // 4, (core % 4) * QS
        # rotate so this core's query strip is rows 0..QS-1 (attention and
        # group stats are permutation-invariant over tokens), then go
        # channel-major for direct DMA into the resident XT tiles
        xr = np.roll(xf[b], -qoff, axis=0)
        in_maps.append({
            **common,
            "xt": np.ascontiguousarray(xr.T.astype(BFh)),
            "xres": np.ascontiguousarray(xf[b, qoff:qoff + QS]),
        })
    return in_maps, fold_q


def kernel(x, gamma, beta, wq, bq, wk, bk, wv, bv, wp, bp):
    in_maps, fold_q = prepare_in_maps(x, gamma, beta, wq, bq, wk, bk, wv, bv, wp, bp)
    if fold_q not in _NC_CACHE:
        _NC_CACHE[fold_q] = _build(fold_q)
    nc = _NC_CACHE[fold_q]
    res = run_bass_kernel_spmd(nc, in_maps, list(range(NCORES)))
    out = np.empty((B, T, C), np.float32)
    for core in range(NCORES):
        b, qoff = core // 4, (core % 4) * QS
        out[b, qoff:qoff + QS] = res.results[core]["out"]
    return out.reshape(B, H, W, C)
